# revision 17
# baseline (speedup 1.0000x reference)
"""Trainium2 Bass kernel for NeuroplasticLlama block-sparse adapter (moe_routing).

Contract: kernel(**inputs) takes FULL unsharded inputs (as produced by
setup_inputs) and returns the FULL [4, 4096, 4096] float32 output.

Strategy (data/sequence parallel over 8 cores, 2048 tokens each):
  - Each core's 2048 contiguous tokens belong to exactly one batch, so the
    task embedding contributes only per-core constant bias vectors
    (te @ A, te @ W2) -- h = x + te is never materialized.
  - The whole routed computation is made dense:
      scores s[t,n] = x @ (Wp @ centers.T)[:,n] + const_n   (per-token shift
        dropped; softmax over top-k and the top-k set are shift invariant)
      top-3 selection via threshold = 3rd max (3 rounds of max + mask-out)
      gates g[t,n] = exp(s - max) * (s >= thr3) / sum(...)
      z[t,:] (all 512 block-rank pairs) = x @ A_all  (dense)
      zg = z * expand4(g);  delta = block-diag(Bm) matmul;  out = x + delta
  - x is fed pre-transposed [H, tokens] so the H contraction sits on the
    partition dim; output is produced transposed and un-transposed on host.

v3 design (from v2 trace analysis: PE 67% busy w/ HAM cold stretches, DVE
54%, DMA 170 GB/s avg):
  - x in / y out bf16 (36 MB HBM traffic per core vs 69 f32).
  - scores fused into the fp8 DoubleRow stream as a 5th 128-row chunk
    (W2 pre-scaled x16 into e4m3 normal range, un-scaled by the PSUM-evac
    activation's `scale`); A pre-scaled x8 the same way.
  - delta matmuls row-tiled (tile_position): the block-diag stationary has
    16 live rows per h-chunk, so 4 h-chunks ride the four 32-row groups of
    the PE array concurrently -> ~4x fewer PE-serialized delta matmuls.
  - gating tiles f16 (2x DVE rate, 1 cyc/col transposes, finer mantissa
    than bf16 so threshold ties stay rare).
  - residual adds read delta straight from PSUM, alternating DVE/GpSimd.
  - software pipelining: macrotile mt's gx/delta/adds/stores are issued
    between the z-matmul chains of macrotile mt+1 so the PE never drains
    (HAM re-throttles to 1.2 GHz after ~3.4us idle).
  - numerics vs reference (numpy model): rel-L2 ~2.1e-3 (budget 2e-2).
"""

import sys

if "/opt/trn_rl_repo" not in sys.path:
    sys.path.insert(0, "/opt/trn_rl_repo")

import numpy as np
import ml_dtypes

H = 4096
NB = 128
BLK = 32
R = 4
B = 4
S = 4096
NCORES = 8
TPC = (B * S) // NCORES  # tokens per core = 2048
T = 512                  # tokens per macrotile
NMT = TPC // T           # 4 macrotiles per core
NKT = H // 128           # 32 k-tiles over the hidden dim
BIG = 30000.0            # f16-safe "+inf" for the top-3 mask-out
SW = 16.0                # fp8 pre-scale for W2 (scores weights)
AS = 8.0                 # fp8 pre-scale for A  (z weights)

TRACE = False            # set by test.py for profiling runs
TRACE_DIR = None
LAST_RESULT = None       # BassKernelResults of the last run

_COMPILED = None


def _build():
    import concourse.bacc as bacc
    import concourse.tile as tile
    from concourse import mybir, masks

    f32 = mybir.dt.float32
    f16 = mybir.dt.float16
    bf16 = mybir.dt.bfloat16
    f8 = mybir.dt.float8e4
    AF = mybir.ActivationFunctionType
    AL = mybir.AluOpType
    AX = mybir.AxisListType
    DR = mybir.MatmulPerfMode.DoubleRow

    nc = bacc.Bacc("TRN2", target_bir_lowering=False, debug=False,
                   num_devices=NCORES)

    xt_d = nc.dram_tensor("xt", [H, TPC], bf16, kind="ExternalInput")
    ah_d = nc.dram_tensor("ah", [128, 5 * NKT * 128], f8, kind="ExternalInput")
    bpk_d = nc.dram_tensor("bpk", [128, NKT * 128], bf16, kind="ExternalInput")
    e_d = nc.dram_tensor("e", [128, 512], f16, kind="ExternalInput")
    bias_d = nc.dram_tensor("bias", [128, 5], f32, kind="ExternalInput")
    yt_d = nc.dram_tensor("yt", [H, TPC], bf16, kind="ExternalOutput")

    xt_ap = xt_d.ap()
    yt_ap = yt_d.ap()

    with tile.TileContext(nc) as tc:
        from contextlib import ExitStack
        with ExitStack() as ctx:
            cpool = ctx.enter_context(tc.tile_pool(name="consts", bufs=1))
            xpool = ctx.enter_context(tc.tile_pool(name="xg", bufs=18))
            xbpool = ctx.enter_context(tc.tile_pool(name="xb", bufs=12))
            zpool = ctx.enter_context(tc.tile_pool(name="zb", bufs=9))
            gpool = ctx.enter_context(tc.tile_pool(name="gate", bufs=3))
            spool = ctx.enter_context(tc.tile_pool(name="scal", bufs=6))
            pp = ctx.enter_context(tc.tile_pool(name="ps", bufs=2, space="PSUM"))

            # ---- persistent constants (gpsimd DMA queue) ----
            az = []
            for q in range(5):
                t_az = cpool.tile([128, NKT * 128], f8, name=f"az{q}",
                                  tag=f"az{q}")
                nc.gpsimd.dma_start(
                    t_az[:], ah_d.ap()[:, q * NKT * 128:(q + 1) * NKT * 128])
                az.append(t_az)
            bpk = cpool.tile([128, NKT * 128], bf16, name="bpk", tag="bpk")
            nc.gpsimd.dma_start(bpk[:], bpk_d.ap()[:])
            esb = cpool.tile([128, 512], f16, name="esb", tag="esb")
            nc.gpsimd.dma_start(esb[:], e_d.ap()[:])
            bias = cpool.tile([128, 5], f32, name="bias", tag="bias")
            nc.gpsimd.dma_start(bias[:], bias_d.ap()[:])
            ident = cpool.tile([128, 128], f16, name="ident", tag="ident")
            masks.make_identity(nc, ident[:])

            NTS = T // 128  # token sub-tiles per macrotile

            state = {}  # per-mt tiles carried into the next iteration

            def load_and_cast(mt):
                t0 = mt * T
                xgs, xbs = [], []
                for g in range(8):
                    xg = xpool.tile([128, 4 * T], bf16, name="xg", tag="xg")
                    nc.sync.dma_start(
                        xg.rearrange("p (hl t) -> p hl t", hl=4),
                        xt_ap[g * 512:(g + 1) * 512, t0:t0 + T]
                        .rearrange("(hl p) t -> p hl t", p=128),
                    )
                    xgs.append(xg)
                    xb = xbpool.tile([128, 4 * T], f8, name="xb", tag="xb")
                    nc.scalar.copy(xb[:], xg[:])
                    xbs.append(xb)
                return xgs, xbs

            def chain(mt, q, xbs):
                """One fp8 DoubleRow accumulation chain: q<4 -> z chunk,
                q==4 -> scores. Returns the evacuated SBUF tile."""
                zp = pp.tile([128, T], f32, space="PSUM", name="zp", tag="zp")
                for k2 in range(NKT // 2):
                    g, hl = k2 // 2, (k2 % 2) * 2
                    nc.tensor.matmul(
                        zp[:],
                        az[q][:, k2 * 256:(k2 + 1) * 256]
                        .rearrange("p (two m) -> p two m", two=2),
                        xbs[g][:, hl * T:(hl + 2) * T]
                        .rearrange("p (two t) -> p two t", two=2),
                        start=(k2 == 0), stop=(k2 == NKT // 2 - 1),
                        perf_mode=DR,
                    )
                if q == 4:
                    s_sb = gpool.tile([128, T], f16, name="s_sb", tag="s_sb")
                    nc.scalar.activation(s_sb[:], zp[:], AF.Identity,
                                         bias=bias[:, 4:5], scale=1.0 / SW)
                    return s_sb
                zb = zpool.tile([128, T], bf16, name="zb", tag="zb")
                nc.scalar.activation(zb[:], zp[:], AF.Identity,
                                     bias=bias[:, q:q + 1], scale=1.0 / AS)
                return zb

            def gating(mt, s_sb):
                """Transpose scores to token-major, top-3 softmax on DVE,
                transpose gates back. Returns gt_sb [block, T] f16."""
                gt_sb = gpool.tile([128, T], f16, name="gt_sb", tag="gt_sb")
                stns = []
                for ts in range(NTS):
                    # [128,1024] f16 = same 2KB/partition slot as the f32 gx
                    # tiles -> shares the "gx" PSUM tag
                    s_ps = pp.tile([128, 1024], f16, space="PSUM", name="s_ps",
                                   tag="gx")
                    nc.tensor.transpose(s_ps[:, 0:128],
                                        s_sb[:, ts * 128:(ts + 1) * 128],
                                        ident[:])
                    stn = gpool.tile([128, 128], f16, name="stn", tag="stn",
                                     bufs=NTS + 1)
                    nc.scalar.copy(stn[:], s_ps[:, 0:128])
                    stns.append(stn)
                ggs = []
                for ts in range(NTS):
                    # mask chain on GpSimd (SBUF-only ops; DVE is loaded with
                    # the PSUM residual adds), softmax tail on DVE
                    stn = stns[ts]
                    r1 = spool.tile([128, 1], f32, name="r1", tag="r1")
                    nc.vector.reduce_max(r1[:], stn[:], axis=AX.X)
                    mb1 = gpool.tile([128, 128], f16, name="mb1", tag="mb1")
                    nc.vector.tensor_scalar(mb1[:], stn[:], r1[:], BIG,
                                            AL.is_ge, AL.mult)
                    s2 = gpool.tile([128, 128], f16, name="s2", tag="s2")
                    nc.vector.tensor_sub(s2[:], stn[:], mb1[:])
                    r2 = spool.tile([128, 1], f32, name="r2", tag="r2")
                    nc.vector.reduce_max(r2[:], s2[:], axis=AX.X)
                    mb2 = gpool.tile([128, 128], f16, name="mb2", tag="mb2")
                    nc.vector.tensor_scalar(mb2[:], s2[:], r2[:], BIG,
                                            AL.is_ge, AL.mult)
                    s3 = gpool.tile([128, 128], f16, name="s3", tag="s3")
                    nc.vector.tensor_sub(s3[:], s2[:], mb2[:])
                    r3 = spool.tile([128, 1], f32, name="r3", tag="r3")
                    nc.vector.reduce_max(r3[:], s3[:], axis=AX.X)
                    nr1 = spool.tile([128, 1], f32, name="nr1", tag="nr1")
                    nc.vector.tensor_scalar_mul(nr1[:], r1[:], -1.0)
                    ex = gpool.tile([128, 128], f16, name="ex", tag="ex")
                    nc.scalar.activation(ex[:], stn[:], AF.Exp, bias=nr1[:],
                                         scale=1.0)
                    em = gpool.tile([128, 128], f16, name="em", tag="em")
                    zs = spool.tile([128, 1], f32, name="zs", tag="zs")
                    nc.vector.scalar_tensor_tensor(em[:], stn[:], r3[:], ex[:],
                                                   AL.is_ge, AL.mult,
                                                   accum_out=zs[:])
                    rz = spool.tile([128, 1], f32, name="rz", tag="rz")
                    nc.vector.reciprocal(rz[:], zs[:])
                    gg = gpool.tile([128, 128], f16, name="gg", tag="gg",
                                    bufs=NTS + 1)
                    nc.vector.tensor_scalar_mul(gg[:], em[:], rz[:])
                    ggs.append(gg)
                for ts in range(NTS):
                    g_ps = pp.tile([128, 1024], f16, space="PSUM", name="g_ps",
                                   tag="gx")
                    nc.tensor.transpose(g_ps[:, 0:128], ggs[ts][:], ident[:])
                    nc.scalar.copy(gt_sb[:, ts * 128:(ts + 1) * 128],
                                   g_ps[:, 0:128])
                return gt_sb

            def gx_mul(mt, q, zbs, gt_sb):
                """Expand gates for chunk q and scale zb in place."""
                gx = pp.tile([128, 512], f32, space="PSUM", name="gx",
                             tag="gx")
                nc.tensor.matmul(gx[:, 0:T],
                                 esb[:, q * 128:(q + 1) * 128],
                                 gt_sb[:],
                                 start=True, stop=True)
                nc.vector.tensor_mul(zbs[q][:], zbs[q][:], gx[:, 0:T])

            def delta_pass(mt, q, p, zbs, xgs):
                """Row-tiled delta for xg group g=2q+p (h-chunks q*8+4p..+4):
                4 concurrent K=32 matmuls on the four 32-row PE groups, adds
                alternating DVE/GpSimd, then store the group."""
                t0 = mt * T
                g = 2 * q + p
                dps = []
                for i in range(4):
                    hc = q * 8 + 4 * p + i
                    dp = pp.tile([128, T], f32, space="PSUM", name="dp",
                                 tag="dp", bufs=4)
                    nc.tensor.matmul(dp[:],
                                     bpk[32 * i:32 * i + 32,
                                         hc * 128:(hc + 1) * 128],
                                     zbs[q][32 * i:32 * i + 32, :],
                                     start=True, stop=True,
                                     tile_position=(32 * i, 0))
                    dps.append(dp)
                for i in range(4):
                    xsl = xgs[g][:, i * T:(i + 1) * T]
                    nc.vector.tensor_add(xsl, xsl, dps[i][:])
                nc.gpsimd.dma_start(
                    yt_ap[g * 512:(g + 1) * 512, t0:t0 + T]
                    .rearrange("(hl p) t -> p hl t", p=128),
                    xgs[g].rearrange("p (hl t) -> p hl t", hl=4),
                )

            for mt in range(NMT + 1):
                if mt < NMT:
                    xgs, xbs = load_and_cast(mt)
                    s_sb = chain(mt, 4, xbs)
                    gt_sb = gating(mt, s_sb)
                if mt >= 1:
                    pzbs, pgt, pxgs = (state["zbs"], state["gt_sb"],
                                       state["xgs"])
                    for q in range(4):
                        gx_mul(mt - 1, q, pzbs, pgt)
                if mt < NMT:
                    zbs = []
                    for q in range(4):
                        zbs.append(chain(mt, q, xbs))
                        if mt >= 1:
                            delta_pass(mt - 1, q, 0, pzbs, pxgs)
                            delta_pass(mt - 1, q, 1, pzbs, pxgs)
                    state = {"zbs": zbs, "gt_sb": gt_sb, "xgs": xgs}
                else:
                    for q in range(4):
                        delta_pass(mt - 1, q, 0, pzbs, pxgs)
                        delta_pass(mt - 1, q, 1, pzbs, pxgs)

    nc.compile()
    return nc


def _rowperm():
    """zrow permutation: within q-chunk, h-chunk j's 16 (mblk, r) rows sit at
    partitions (j%4)*32 + (j//4)*16 + mblk*4 + r (row-group tiling layout).
    Returns perm[128] mapping new position -> old (j*16 + mblk*4 + r)."""
    perm = np.empty(128, np.int64)
    for j in range(8):
        for mblk in range(4):
            for r in range(R):
                new = (j % 4) * 32 + (j // 4) * 16 + mblk * 4 + r
                perm[new] = j * 16 + mblk * 4 + r
    return perm


def _prep_consts(task_emb, task_ids, Wp, bp, centers, A, Bm, adapter_scale):
    scale = float(np.asarray(adapter_scale))
    A_all = np.ascontiguousarray(
        A.transpose(1, 0, 2).reshape(H, NB * R).astype(np.float32))
    W2 = (Wp @ centers.T).astype(np.float32)                     # [H, 128]
    perm = _rowperm()

    # permute A's columns within each 128-col chunk to the row-tiled order
    A_p = A_all.reshape(H, 4, 128)[:, :, perm].reshape(H, NB * R)

    # ah: 5 fp8 chunks; q<4: [p, hc, m] = A_p[hc*128+p, q*128+m] * AS;
    # q==4: W2 * SW with the same [p, hc, m] transform.
    AW = np.concatenate([A_p * AS, W2 * SW], axis=1)             # [H, 640]
    ah = (AW.reshape(NKT, 128, 5, 128).transpose(1, 2, 0, 3)
          .reshape(128, 5 * NKT * 128).astype(ml_dtypes.float8_e4m3))
    ah = np.ascontiguousarray(ah)

    # block-diag up-projection in the row-tiled layout
    bpk = np.zeros((128, NKT * 128), np.float32)
    for hc in range(NKT):
        j = hc % 8
        for mblk in range(4):
            n = hc * 4 + mblk
            for r in range(R):
                row = (j % 4) * 32 + (j // 4) * 16 + mblk * 4 + r
                bpk[row, hc * 128 + mblk * 32: hc * 128 + mblk * 32 + 32] = \
                    Bm[n, r, :] * scale
    bpk = bpk.astype(ml_dtypes.bfloat16)

    # gate-expand matrix in the permuted order: e[p, q*128+m] = 1 iff the
    # zrow at position m of chunk q belongs to block p.
    blk_of = (np.arange(512) // 4).reshape(4, 128)[:, perm].reshape(512)
    e_np = (np.arange(128)[:, None] == blk_of[None, :]) \
        .astype(np.float16)
    e_np = np.ascontiguousarray(e_np)

    sconst = (bp @ centers.T - 0.5 * (centers ** 2).sum(-1)).astype(np.float32)

    biases = []
    for c in range(NCORES):
        te = task_emb[int(np.asarray(task_ids)[c // 2])].astype(np.float32)
        b5 = np.empty((128, 5), np.float32)
        zoff = te @ A_p                                          # [512]
        for q in range(4):
            b5[:, q] = zoff[q * 128:(q + 1) * 128]
        b5[:, 4] = te @ W2 + sconst
        biases.append(np.ascontiguousarray(b5))
    return ah, bpk, e_np, biases


def kernel(x, task_ids, task_emb, Wp, bp, centers, A, Bm, adapter_scale):
    global _COMPILED, LAST_RESULT
    from concourse import bass_utils

    x = np.asarray(x, dtype=np.float32)
    task_ids = np.asarray(task_ids)
    task_emb = np.asarray(task_emb, dtype=np.float32)
    Wp = np.asarray(Wp, dtype=np.float32)
    bp = np.asarray(bp, dtype=np.float32)
    centers = np.asarray(centers, dtype=np.float32)
    A = np.asarray(A, dtype=np.float32)
    Bm = np.asarray(Bm, dtype=np.float32)

    if _COMPILED is None:
        _COMPILED = _build()
    nc = _COMPILED

    ah, bpk, e_np, biases = _prep_consts(
        task_emb, task_ids, Wp, bp, centers, A, Bm, adapter_scale)

    xf = x.reshape(B * S, H)
    in_maps = []
    for c in range(NCORES):
        xtc = np.ascontiguousarray(
            xf[c * TPC:(c + 1) * TPC].T.astype(ml_dtypes.bfloat16))
        in_maps.append({"xt": xtc, "ah": ah, "bpk": bpk,
                       "e": e_np, "bias": biases[c]})

    kwargs = {}
    if TRACE:
        kwargs = dict(trace=True, tmpdir=TRACE_DIR)
    res = bass_utils.run_bass_kernel_spmd(
        nc, in_maps, core_ids=list(range(NCORES)), **kwargs)
    LAST_RESULT = res

    out = np.empty((B * S, H), np.float32)
    for c in range(NCORES):
        out[c * TPC:(c + 1) * TPC] = res.results[c]["yt"].T.astype(np.float32)
    return out.reshape(B, S, H)
